# revision 62
# baseline (speedup 1.0000x reference)
"""Multi-head causal attention on 8 Trainium2 NeuronCores.

Sharding: data-parallel over batch (4) x tensor-parallel over heads (2 groups
of 8 heads). Each core computes a partial output [T, C] for one batch element
using its 8 heads; the host sums the two partials per batch element (the
"all-reduce after out_proj" done during unshard).

Structure: a single loop over 4 token chunks.  Unit t projects chunk t
(Q/K/V), runs attention for query chunk j=t (whose keys are exactly chunks
0..t, so projection and attention overlap and the exp stream on the Scalar
engine spreads over the whole kernel), then runs softmax-normalization +
out_proj for chunk j=t-1 (software pipelining: the denominator-gather DMA +
reciprocal_approx_fast of chunk j run during unit j+1's matmuls, so the PE
never waits on them; out_proj doubles as PE filler under the last recip
chain).  Causal raggedness: for query chunk j, diagonal key block kb=4j+m
streams only the N = 512-128m live query columns, exp covers only the live
range, and the causal mask is one [128,128] triangle multiply per diagonal
block.  The kb loop is software-pipelined one deep (AV of kb-1 emitted
after the scores of kb) so the in-order PE queue never waits on exp/mask.
All DMAs use host-prearranged contiguous layouts; the output leaves as
bf16 (the host sums the two head-group partials in f32).

Engine budget per core (theory): PE 549k matmul rows ~= 229 us (projection
197k, ragged scores 139k, ragged AV 139k, out_proj 66k, norm bcast 8k),
Scalar exp ~140k cols ~= 125 us, Vector copies/masks/mults ~= 110 us.
Measured: ~287 us vs 364 us for the phase-separated baseline.

Per-core layouts (no on-device transposes needed):
  inputs: xT [C, T] (x[b] transposed on host), Wq*0.125/Wk/Wv [C, 512],
          Wo [512, C], triangle mask [128, 128] bf16, sel [8, 512] f32r.
  QT = (Wq/8)^T @ x^T  [512, T]   (lhsT = Wq chunk, rhs = xT chunk)
  KT = Wk^T @ x^T      [512, T]
  V  = x @ Wv          [T, 512]   ones-augmented as vaug [T, 8 heads, 65]
  per head-pair p, query-chunk j, key-block kb (ragged causal):
     sT  = K_h[kb]^T @ Q_h[:, live]      [128, <=512] PSUM per half
     p   = exp(sT)  (skip-max softmax: |s| < ~9), one exp per kb tile
     p  *= triangle mask on the leading 128 cols of diagonal blocks
     av += V_aug[kb, h]^T @ p            [65, 512] PSUM; row 64 = denom
  per j: gather the 8 denom rows, reciprocal_approx_fast, sel-matmul
  broadcast, normalize aot in place, out_proj of chunk j -> bf16 DMA out.
"""

import numpy as np
import ml_dtypes

_BF = ml_dtypes.bfloat16

import concourse.bass as bass
import concourse.bacc as bacc
import concourse.mybir as mybir
import concourse.tile as tile
from concourse import bass_utils

F32 = mybir.dt.float32
F32R = mybir.dt.float32r
BF16 = mybir.dt.bfloat16

B, T, C = 4, 2048, 1024
H, Dh = 16, 64
G = 2                 # head groups (tensor parallel)
HPG = H // G          # heads per group
GC = HPG * Dh         # group channels = 512
N_CORES = 8
TC = 512              # token chunk (projection and query chunks)
KB = 128              # key block
N_TC = T // TC        # 4
N_KB = T // KB        # 16
N_CC = C // 128       # contraction chunks over C = 8
N_GCB = GC // 128     # chan blocks in a group = 4


def build_program():
    nc = bacc.Bacc("TRN2", target_bir_lowering=False, debug=False)

    # all inputs host-prearranged to the on-chip layout so every DMA line
    # is contiguous (2-8 KB per partition)
    xT = nc.dram_tensor("xT", [N_TC, 128, N_CC, TC], BF16, kind="ExternalInput").ap()
    wq = nc.dram_tensor("wq", [128, N_GCB, N_CC, 128], BF16, kind="ExternalInput").ap()
    wk = nc.dram_tensor("wk", [128, N_GCB, N_CC, 128], BF16, kind="ExternalInput").ap()
    wv = nc.dram_tensor("wv", [128, N_CC, GC], BF16, kind="ExternalInput").ap()
    wo = nc.dram_tensor("wo", [128, N_GCB, C], BF16, kind="ExternalInput").ap()
    mask_in = nc.dram_tensor("mask", [KB, KB], BF16, kind="ExternalInput").ap()
    sel_in = nc.dram_tensor("sel", [8, 8 * Dh], BF16, kind="ExternalInput").ap()
    out = nc.dram_tensor("out", [T, C], BF16, kind="ExternalOutput").ap()

    with tile.TileContext(nc) as tc:
        with (
            tc.tile_pool(name="persist", bufs=1) as pp,
            tc.tile_pool(name="x_pool", bufs=3) as xp,
            tc.tile_pool(name="probs", bufs=6) as prp,
            tc.tile_pool(name="outs", bufs=3) as otp,
            tc.tile_pool(name="dc_pool", bufs=2) as dcp,
            tc.tile_pool(name="pj_psum", bufs=2, space="PSUM") as pjp,
            tc.tile_pool(name="sc_psum", bufs=2, space="PSUM") as scp,
            tc.tile_pool(name="av_psum", bufs=2, space="PSUM") as avp,
        ):
            qt = pp.tile([128, N_GCB, T], BF16)        # QT (chan%128, chan//128, tok)
            kt = pp.tile([128, N_GCB, T], BF16)
            vaug = pp.tile([128, N_KB, HPG, Dh + 1], BF16)
            aot = pp.tile([128, N_GCB, T], BF16)       # attn_outT
            msk = pp.tile([128, KB], BF16)
            sel = pp.tile([8, 8 * Dh], BF16)
            # softmax denominator staging: slot idx8 at partition
            # 32*(idx8//3), column idx8%3 (engine APs start at 0/32/64);
            # rewritten every query chunk, WAR-protected by the gather DMA
            dens = pp.tile([65, 3, TC], F32)

            wqs = pp.tile([128, N_GCB, N_CC, 128], BF16)
            wks = pp.tile([128, N_GCB, N_CC, 128], BF16)
            wvs = pp.tile([128, N_CC, GC], BF16)
            wos = pp.tile([128, N_GCB, C], BF16)

            xts = []
            for t in range(N_TC):
                xts.append(
                    xp.tile([128, N_CC, TC], BF16, tag="xt", name=f"xt{t}")
                )

            # startup DMA order = need order; kc-split first slices so the
            # first half-groups can start after ~0.6 MB
            nc.sync.dma_start(wqs[:, 0, 0:4], wq[:, 0, 0:4])
            nc.sync.dma_start(xts[0][:, 0:4], xT[0][:, 0:4])
            nc.sync.dma_start(wqs[:, 1, 0:4], wq[:, 1, 0:4])
            nc.sync.dma_start(xts[0][:, 4:], xT[0][:, 4:])
            nc.sync.dma_start(wqs[:, 0, 4:], wq[:, 0, 4:])
            nc.sync.dma_start(wqs[:, 1, 4:], wq[:, 1, 4:])
            for oc in range(2, N_GCB):
                nc.sync.dma_start(wqs[:, oc], wq[:, oc])
            for oc in range(N_GCB):
                nc.sync.dma_start(wks[:, oc], wk[:, oc])
            nc.sync.dma_start(wvs[:], wv)
            nc.sync.dma_start(msk[:], mask_in)
            nc.sync.dma_start(sel[:], sel_in)
            nc.sync.dma_start(wos[:], wo)
            nc.vector.memset(vaug[:, :, :, Dh:], 1.0)

            recs = {}

            def gather_recip(j):
                # short chain after the last pair's dens copies; the rest
                # overlaps following matmuls
                dcomp = dcp.tile([8, TC], F32, tag="dc", name="dc")
                for b3 in range(3):
                    lo, n = 3 * b3, min(3, 8 - 3 * b3)
                    nc.sync.dma_start(
                        dcomp[lo:lo + n, :], dens[32 * b3:32 * b3 + 1, :n, :]
                    )
                scr = dcp.tile([8, TC], F32, tag="scr", name="scr")
                rec = dcp.tile([8, TC], BF16, tag="rec", name="rec")
                nc.vector.reciprocal_approx_fast(scr[:], dcomp[:])
                nc.vector.tensor_copy(rec[:], scr[:])
                return rec

            def norm(j, rec):
                qslc = slice(j * TC, (j + 1) * TC)
                for p in range(HPG // 2):
                    # both halves of the pair in one bcast matmul + one mult
                    bc = pjp.tile([128, TC], F32, tag="pj", name="bc")
                    nc.tensor.matmul(
                        bc[:],
                        sel[:, 2 * p * Dh:(2 * p + 2) * Dh],
                        rec[:],
                        start=True, stop=True,
                    )
                    nc.vector.tensor_mul(
                        aot[:, p, qslc], aot[:, p, qslc], bc[:]
                    )

            def outproj(j, tbs):
                for tb in tbs:
                    ot = otp.tile([128, C], BF16, tag="ot", name="ot")
                    # both oc halves in one 2-bank tile on the (idle-here)
                    # sc ring: decouples out_proj from the projection
                    # ring's WAR chain
                    ps = scp.tile([128, 2, TC], F32, tag="sc", name="op")
                    for oc in range(C // TC):
                        for cc in range(N_GCB):
                            nc.tensor.matmul(
                                ps[:, oc, :],
                                aot[:, cc, tb * 128:(tb + 1) * 128],
                                wos[:, cc, oc * TC:(oc + 1) * TC],
                                start=(cc == 0),
                                stop=(cc == N_GCB - 1),
                            )
                    # scalar helps only on the final chunk's copies,
                    # when its exp stream has already finished
                    if j == N_TC - 1:
                        nc.scalar.copy(ot[:, 0:TC], ps[:, 0, :])
                        nc.vector.tensor_copy(ot[:, TC:], ps[:, 1, :])
                    else:
                        nc.vector.tensor_copy(
                            ot.rearrange("p (o n) -> p o n", o=2), ps[:]
                        )
                    nc.sync.dma_start(out[tb * 128:(tb + 1) * 128, :], ot[:])

            for t in range(N_TC):
                # -------- phase 2 unit: project token chunk t ------------
                xt = xts[t]
                if t + 1 < N_TC:
                    nc.sync.dma_start(xts[t + 1][:], xT[t + 1])
                def qk_group(w_s, dst, oc, ps=None, kcs=range(N_CC)):
                    if ps is None:
                        ps = pjp.tile([128, TC], F32, tag="pj", name="pj")
                    for kc in kcs:
                        nc.tensor.matmul(
                            ps[:],
                            w_s[:, oc, kc, :],
                            xt[:, kc, :],
                            start=(kc == 0),
                            stop=(kc == N_CC - 1),
                        )
                    if kcs[-1] == N_CC - 1:
                        dslc = dst[:, oc, t * TC:(t + 1) * TC]
                        # scalar takes Q copies only in exp-light units 0/1
                        if w_s is wqs and t < 2:
                            nc.scalar.copy(dslc, ps[:])
                        else:
                            nc.vector.tensor_copy(dslc, ps[:])
                    return ps

                if t == 0:
                    # unit 0: Q oc0/oc1 interleaved as kc-half-groups in DMA
                    # arrival order, then the rest of Q, then all of K
                    ps0 = qk_group(wqs, qt, 0, kcs=range(4))
                    ps1 = qk_group(wqs, qt, 1, kcs=range(4))
                    qk_group(wqs, qt, 0, ps=ps0, kcs=range(4, N_CC))
                    qk_group(wqs, qt, 1, ps=ps1, kcs=range(4, N_CC))
                    for oc in range(2, N_GCB):
                        qk_group(wqs, qt, oc)
                    for oc in range(N_GCB):
                        qk_group(wks, kt, oc)
                else:
                    for oc in range(N_GCB):
                        qk_group(wqs, qt, oc)
                        qk_group(wks, kt, oc)
                for tb in range(TC // 128):  # V token blocks
                    ps = pjp.tile([128, GC], F32, tag="pj", name="pj")
                    for kc in range(N_CC):
                        nc.tensor.matmul(
                            ps[:],
                            xt[:, kc, tb * 128:(tb + 1) * 128],
                            wvs[:, kc, :],
                            start=(kc == 0),
                            stop=(kc == N_CC - 1),
                        )
                    nc.vector.tensor_copy(
                        vaug[:, t * 4 + tb, :, :Dh],
                        ps.rearrange("p (h d) -> p h d", h=HPG),
                    )

                # -------- phase 3 unit: attention for query chunk j=t ----
                j = t
                qslc = slice(j * TC, (j + 1) * TC)
                for p in range(HPG // 2):    # head pairs: rows 0:64 / 64:128
                    avs = [
                        avp.tile([Dh + 1, TC], F32, tag="av", name=f"av{i}")
                        for i in range(2)
                    ]
                    nkb = 4 * j + 4

                    def emit_av(kb, pr, c0, w):
                        for half in range(2):
                            src = (
                                pr[:, c0:TC] if half == 0
                                else pr[:, TC:TC + w]
                            )
                            nc.tensor.matmul(
                                avs[half][:, c0:],
                                vaug[:, kb, 2 * p + half, :],
                                src,
                                start=(kb == 0),
                                stop=(kb == nkb - 1),
                            )

                    # kb loop software-pipelined one deep: AV for kb-1 is
                    # emitted after the scores of kb, so the PE never waits
                    # on exp/mask of the block it is about to consume
                    pend = None
                    for kb in range(nkb):
                        m = kb - 4 * j       # >=0 on diagonal blocks
                        c0 = m * 128 if m > 0 else 0
                        w = TC - c0          # live query columns per half
                        # both heads' score tiles packed [c0:512 | 512:512+w]
                        # in one 2-bank PSUM tile -> single exp op per kb
                        sc = scp.tile([128, 2 * TC], F32, tag="sc", name="sc")
                        for half in range(2):
                            p0 = half * Dh
                            dst = (
                                sc[:, c0:TC] if half == 0
                                else sc[:, TC:TC + w]
                            )
                            nc.tensor.matmul(
                                dst,
                                kt[p0:p0 + Dh, p, kb * KB:(kb + 1) * KB],
                                qt[p0:p0 + Dh, p, j * TC + c0:(j + 1) * TC],
                                start=True,
                                stop=True,
                            )
                        pr = prp.tile([128, 2 * TC], BF16, tag="pr", name="pr")
                        nc.scalar.activation(
                            pr[:, c0:TC + w], sc[:, c0:TC + w],
                            mybir.ActivationFunctionType.Exp,
                        )
                        if m >= 0:
                            # causal triangle on the leading 128 live cols
                            for half in range(2):
                                base = c0 if half == 0 else TC
                                nc.vector.tensor_mul(
                                    pr[:, base:base + KB],
                                    pr[:, base:base + KB],
                                    msk[:],
                                )
                        if pend is not None:
                            emit_av(*pend)
                        pend = (kb, pr, c0, w)
                    emit_av(*pend)
                    # denominator rows first: they head the recip chain.
                    # On the unit's last pair, scalar takes one copy (its
                    # next exp is a unit away) and the aot copies are
                    # deferred past the recip chain — both shorten the
                    # vector path from last AV to the bcast matmul.
                    last = p == HPG // 2 - 1
                    for half in range(2):
                        idx8 = 2 * p + half
                        db, dc = 32 * (idx8 // 3), idx8 % 3
                        eng = nc.scalar if (last and half == 0) else nc.vector
                        if eng is nc.scalar:
                            nc.scalar.copy(
                                dens[db:db + 1, dc, :], avs[half][Dh:Dh + 1, :]
                            )
                        else:
                            nc.vector.tensor_copy(
                                dens[db:db + 1, dc, :], avs[half][Dh:Dh + 1, :]
                            )
                    if not last:
                        for half in range(2):
                            p0 = half * Dh
                            nc.vector.tensor_copy(
                                aot[p0:p0 + Dh, p, qslc], avs[half][:Dh, :]
                            )
                    else:
                        deferred = avs

                rec = gather_recip(t)
                for half in range(2):  # deferred last-pair aot copies
                    p0 = half * Dh
                    nc.vector.tensor_copy(
                        aot[p0:p0 + Dh, HPG // 2 - 1, qslc],
                        deferred[half][:Dh, :],
                    )
                recs[t] = rec
                if t >= 1:
                    norm(t - 1, recs.pop(t - 1))
                    outproj(t - 1, range(4 * t - 4, 4 * t))

            norm(N_TC - 1, recs.pop(N_TC - 1))
            outproj(N_TC - 1, range(4 * N_TC - 4, 4 * N_TC))

    nc.compile()
    return nc


_CACHE = {}


def _make_mask():
    m = np.zeros((KB, KB), np.float32)
    for dk in range(KB):
        m[dk, dk:] = 1.0
    return m.astype(_BF)


def _make_sel():
    s = np.zeros((8, 8 * Dh), np.float32)
    for i in range(8):
        s[i, i * Dh:(i + 1) * Dh] = 1.0
    return s.astype(_BF)


def _arr_qk(w):
    # [C, GC] -> [128, N_GCB, N_CC, 128]: w[kc*128+p, oc*128+c] -> [p,oc,kc,c]
    return np.ascontiguousarray(
        w.reshape(N_CC, 128, N_GCB, 128).transpose(1, 2, 0, 3)
    )


def _arr_kcmaj(w):
    # [C, N] -> [128, N_CC, N]: w[kc*128+p, n] -> [p, kc, n]
    return np.ascontiguousarray(w.reshape(N_CC, 128, -1).transpose(1, 0, 2))


def make_in_maps(x, W_qkv, W_out):
    mask = _make_mask()
    sel = _make_sel()
    in_maps = []
    for core in range(N_CORES):
        b, g = divmod(core, G)
        cs = slice(g * GC, (g + 1) * GC)
        xTb = x[b].T.astype(_BF)  # [C, T]
        in_maps.append({
            # [C,T] -> [N_TC, 128, N_CC, TC]: xT[kc*128+p, t*TC+n]
            "xT": np.ascontiguousarray(
                xTb.reshape(N_CC, 128, N_TC, TC).transpose(2, 1, 0, 3)
            ),
            "wq": _arr_qk((W_qkv[:, cs] * 0.125).astype(_BF)),
            "wk": _arr_qk(W_qkv[:, C + g * GC:C + (g + 1) * GC].astype(_BF)),
            "wv": _arr_kcmaj(
                W_qkv[:, 2 * C + g * GC:2 * C + (g + 1) * GC].astype(_BF)
            ),
            # [GC, C] -> [128, N_GCB, C]
            "wo": np.ascontiguousarray(
                W_out[cs, :].astype(_BF).reshape(N_GCB, 128, C).transpose(1, 0, 2)
            ),
            "mask": mask,
            "sel": sel,
        })
    return in_maps


def kernel(x, W_qkv, W_out):
    x = np.ascontiguousarray(np.asarray(x, dtype=np.float32))
    W_qkv = np.asarray(W_qkv, dtype=np.float32)
    W_out = np.asarray(W_out, dtype=np.float32)

    if "nc" not in _CACHE:
        _CACHE["nc"] = build_program()
    nc = _CACHE["nc"]

    in_maps = make_in_maps(x, W_qkv, W_out)
    res = bass_utils.run_bass_kernel_spmd(nc, in_maps, core_ids=list(range(N_CORES)))

    out = np.empty((B, T, C), np.float32)
    for b in range(B):
        out[b] = res.results[G * b]["out"].astype(np.float32)
        for g in range(1, G):
            out[b] += res.results[G * b + g]["out"].astype(np.float32)
    return out
